# revision 59
# baseline (speedup 1.0000x reference)
"""PointPillarScatter on 8 TRN2 cores via PE one-hot matmul, fp16.

Scatter -> dense-matmul transform, one PE op per chunk of 8 canvas
tiles (tile = 128 consecutive canvas columns):

  out[64h+f, 512c + n] = sum_k F[k, 64h+f] * P[k, 512c + n]

  lhsT = F [K=128, M=128]   stationary, block-diag: slot rows [0,64)
         (pool for the 4 "half A" tiles) carry feats in cols [0,64);
         rows [64,128) (half B pool) in cols [64,128).
  rhs  = P [K=128, N=512]   one-hot, built on DVE by a single
         tensor_scalar is_equal against a device-built iota:
         P[k, n] = (colof[k] == n), colof = 128*jj + cc encodes both
         the tile-within-half jj and the column cc; empty slots -1.
         (tensor_scalar, not a broadcast tensor_tensor: the [128,1]
         scalar operand is dtype-exempt, so the op runs in the DVE
         16-bit 2x mode.)

The matmul runs with is_transpose=True: P is a partial permutation
(at most one 1 per row AND per column), which is exactly the
generalized-transpose selection the PE transpose datapath performs,
and transpose mode writes PSUM in the lhsT dtype -- fp16.  fp16 PSUM
(a) halves PSUM bank usage (a [128, 1024] pair of chunks fits one
bank; transpose-mode writes may share a bank without the TRN2
multi-accumulation-group fault) and (b) makes the PSUM->SBUF copies
2-byte packed reads, so the DVE copies them in its 2x mode (~0.6us
per pair vs ~1.0us on Act).  Copies split ~2:1 Act:DVE; 10 dummy
warm-up matmuls ramp the PE p-state while input DMAs land.

Slots are pooled per (chunk, half): 4 tiles share 64 slots, so tile
occupancy only matters in aggregate (mean 43.6, cap 64).  The rare
overflowing group (1 in 1376 for seed 0) is fixed by swapping its
heaviest tile with the lightest tile of the lightest group; the
resulting tile permutation is applied on the host during unshard.

fp16 end-to-end: the one-hot matmul routes fp16 values exactly, so
the only error is the f32->fp16 input cast (~3.6e-4 max relative),
well under the 2e-2 gate, and memory traffic halves vs f32.

History: 194us (f32, per-tile matmuls) -> 88 (fp16 K-stacked chunks)
-> 61 (tensor_scalar 2x is_eq) -> 54 (balance/startup/tail tuning)
-> ~52 (fp16 PSUM via transpose mode).
"""

import numpy as np

import concourse.bass as bass
import concourse.tile as tile
from concourse import mybir
from concourse.bass_utils import run_bass_kernel_spmd

NUM_FEATURES = 64
MAX_CAV = 5
NX, NY = 704, 200
NUM_PIXELS = NY * NX            # 140800
TOTAL = MAX_CAV * NUM_PIXELS    # 704000
N_CORES = 8
CORE_COLS = TOTAL // N_CORES    # 88000 flat columns per core
TILE_COLS = 128
N_TILES = 688                   # 688*128 = 88064 >= 88000
CHUNKS = N_TILES // 8           # 86 chunks of 8 tiles
POOL = 64                       # slots shared by the 4 tiles of one half
OUT_W = CHUNKS * 512            # 44032

_PROG = None


def _split_excess_waits(nc, max_waits=1):
    """Walrus enforces tight per-instruction sync-wait encoding limits. Spill
    surplus waits onto single-wait EventSemaphore nops inserted just before
    the offending instruction on the same engine queue (same semantics:
    engine blocks at the nop, then proceeds)."""
    for blk in nc.main_func.blocks:
        i = 0
        while i < len(blk.instructions):
            inst = blk.instructions[i]
            si = inst.sync_info
            if si is None or len(si.on_wait) <= max_waits:
                i += 1
                continue
            waits = list(si.on_wait)
            keep, spill = waits[-max_waits:], waits[:-max_waits]
            for w in spill:
                nop = mybir.InstEventSemaphore(
                    name=f"I-{nc.next_id()}", ins=[], outs=[]
                )
                nop.engine = inst.engine
                nop.sync_info = mybir.SyncInfo(on_wait=[w], on_update=[])
                nc.register_instruction(nop)
                blk.instructions.insert(i, nop)
                i += 1
            si.on_wait = keep
            inst.sync_info = si
            i += 1


def _build_prog():
    f16 = mybir.dt.float16
    f32 = mybir.dt.float32
    nc = bass.Bass()
    feats = nc.dram_tensor("feats", [128, CHUNKS * 128], f16, kind="ExternalInput")
    colof = nc.dram_tensor("colof", [128, CHUNKS], f32, kind="ExternalInput")
    out = nc.dram_tensor("out", [128, OUT_W], f16, kind="ExternalOutput")

    with tile.TileContext(nc) as tc:
        with (
            tc.tile_pool(name="const", bufs=1) as constp,
            tc.tile_pool(name="pmat", bufs=8) as pmatp,
            tc.tile_pool(name="psum", bufs=6, space="PSUM") as psump,
            tc.tile_pool(name="wpsum", bufs=1, space="PSUM") as wpsump,
            tc.tile_pool(name="stage", bufs=4) as stagep,
        ):
            colof_sb = constp.tile([128, CHUNKS], f32)
            nc.sync.dma_start(colof_sb[:], colof[:])
            # iota built on-device (fp16 is exact for integers <= 2048)
            iota_sb = constp.tile([128, 512], f16)
            nc.gpsimd.iota(
                iota_sb[:], pattern=[[1, 512]], base=0,
                channel_multiplier=0,
                allow_small_or_imprecise_dtypes=True,
            )
            feats_sb = constp.tile([128, CHUNKS * 128], f16)
            lo = 0
            for n in (2, 14, 24, 46):       # chunks per feats DMA piece
                nc.sync.dma_start(
                    feats_sb[:, lo * 128:(lo + n) * 128],
                    feats[:, lo * 128:(lo + n) * 128],
                )
                lo += n

            # PE p-state warm-up: ~3us of back-to-back matmuls on the iota
            # tile while the input DMAs land, so real matmuls start at full
            # clock.  Results are discarded (bank reused by the main loop).
            wps = wpsump.tile([128, 1024], f32, space="PSUM")
            for w in range(10):
                nc.tensor.matmul(
                    out=wps[:, (w % 2) * 512:(w % 2) * 512 + 512],
                    lhsT=iota_sb[:, 0:128],
                    rhs=iota_sb[:],
                    start=True,
                    stop=True,
                )

            groups = [2, 6] + [8] * 9 + [4, 2]
            c0 = 0
            qi = 0
            for n8 in groups:
                st = stagep.tile([128, 512 * n8], f16)
                for p0 in range(0, n8, 2):
                    np_ = min(2, n8 - p0)
                    # fp16 PSUM via the PE transpose path (P is a partial
                    # permutation, i.e. a generalized transpose selector);
                    # fp16 doubles bank capacity: one bank holds both
                    # chunks of a pair (transpose-mode writes share it)
                    ps = psump.tile([128, np_ * 512], f16, space="PSUM")
                    for j in range(p0, p0 + np_):
                        c = c0 + j
                        P = pmatp.tile([128, 512], f16)
                        nc.vector.tensor_scalar(
                            out=P[:],
                            in0=iota_sb[:],
                            scalar1=colof_sb[:, c:c + 1],
                            scalar2=None,
                            op0=mybir.AluOpType.is_equal,
                        )
                        nc.tensor.matmul(
                            out=ps[:, (j - p0) * 512:(j - p0 + 1) * 512],
                            lhsT=feats_sb[:, c * 128:(c + 1) * 128],
                            rhs=P[:],
                            start=True,
                            stop=True,
                            is_transpose=True,
                        )
                    # copy the pair in one contiguous instruction;
                    # fp16 PSUM reads get the DVE 2x mode, so DVE takes more
                    dst = st[:, p0 * 512:(p0 + np_) * 512]
                    src = ps[:]
                    if qi % 3 == 2 and c0 < CHUNKS - 16:
                        nc.vector.tensor_scalar_add(dst, src, 0.0)
                    else:
                        nc.scalar.activation(
                            dst, src, mybir.ActivationFunctionType.Copy
                        )
                    qi += 1
                nc.sync.dma_start(
                    out[:, c0 * 512:(c0 + n8) * 512], st[:]
                )
                c0 += n8
    _split_excess_waits(nc)
    return nc


def _host_prep(voxel_coords, pillar_features):
    vc = voxel_coords.astype(np.int64)
    flat = vc[:, 0] * NUM_PIXELS + vc[:, 2] * NX + vc[:, 3]
    feats = pillar_features.astype(np.float16)
    core = flat // CORE_COLS
    rem = flat - core * CORE_COLS
    t = rem // TILE_COLS            # tile within core, 0..687
    cc = rem - t * TILE_COLS        # column within tile

    in_maps = []
    perms = []
    for cidx in range(N_CORES):
        m = core == cidx
        tc_, cc_, fe_ = t[m], cc[m], feats[m]
        cnt = np.bincount(tc_, minlength=N_TILES)

        # perm[q] = original tile occupying virtual slot q; virtual slot q
        # belongs to chunk q//8, half (q%8)//4, jj q%4.
        perm = np.arange(N_TILES)
        gsum = cnt.reshape(N_TILES // 4, 4).sum(axis=1)
        for _ in range(64):
            gbad = int(np.argmax(gsum))
            if gsum[gbad] <= POOL:
                break
            glight = int(np.argmin(gsum))
            bt = gbad * 4 + int(np.argmax(cnt[perm[gbad * 4:gbad * 4 + 4]]))
            lt = glight * 4 + int(
                np.argmin(cnt[perm[glight * 4:glight * 4 + 4]])
            )
            perm[bt], perm[lt] = perm[lt], perm[bt]
            gsum[gbad] = cnt[perm[gbad * 4:gbad * 4 + 4]].sum()
            gsum[glight] = cnt[perm[glight * 4:glight * 4 + 4]].sum()
        assert gsum.max() <= POOL, f"group overflow: {gsum.max()}"
        perms.append(perm)

        pos = np.empty(N_TILES, np.int64)
        pos[perm] = np.arange(N_TILES)
        q = pos[tc_]                       # virtual tile slot per pillar
        chunk = q // 8
        h = (q % 8) // 4
        jj = q % 4
        grp = chunk * 2 + h                # slot pool id, 0..171

        # slot = rank of pillar within its pool
        order = np.argsort(grp, kind="stable")
        gs = grp[order]
        rank = np.arange(len(gs)) - np.searchsorted(gs, gs, side="left")
        slot = np.empty(len(gs), np.int64)
        slot[order] = rank
        assert slot.max() < POOL

        k = h * POOL + slot
        fa = np.zeros((128, CHUNKS, 2, 64), np.float16)
        fa[k, chunk, h, :] = fe_
        ca = np.full((128, CHUNKS), -1.0, np.float32)
        ca[k, chunk] = (jj * TILE_COLS + cc_).astype(np.float32)
        in_maps.append({
            "feats": fa.reshape(128, CHUNKS * 128),
            "colof": ca,
        })
    return in_maps, perms


def _unshard(core_outs, perms):
    full = np.empty((TOTAL, NUM_FEATURES), np.float32)
    for cidx, o in enumerate(core_outs):       # o: [128, OUT_W] fp16
        v = o.reshape(2, 64, CHUNKS, 4, 128)   # [h, f, chunk, jj, cc]
        v = v.transpose(2, 0, 3, 4, 1)         # [chunk, h, jj, cc, f]
        vt = v.reshape(N_TILES, TILE_COLS, NUM_FEATURES)
        ct = np.empty_like(vt)
        ct[perms[cidx]] = vt
        r = ct.reshape(N_TILES * TILE_COLS, NUM_FEATURES)[:CORE_COLS]
        full[cidx * CORE_COLS:(cidx + 1) * CORE_COLS] = r.astype(np.float32)
    return np.ascontiguousarray(
        full.reshape(MAX_CAV, NUM_PIXELS, NUM_FEATURES)
        .transpose(0, 2, 1)
        .reshape(MAX_CAV, NUM_FEATURES, NY, NX)
    )


def kernel(voxel_coords, pillar_features):
    global _PROG
    if _PROG is None:
        _PROG = _build_prog()
    in_maps, perms = _host_prep(voxel_coords, pillar_features)
    res = run_bass_kernel_spmd(_PROG, in_maps, list(range(N_CORES)))
    return _unshard([r["out"] for r in res.results], perms)


# revision 60
# speedup vs baseline: 1.0018x; 1.0018x over previous
"""PointPillarScatter on 8 TRN2 cores via PE one-hot matmul, fp16.

Scatter -> dense-matmul transform, one PE op per chunk of 8 canvas
tiles (tile = 128 consecutive canvas columns):

  out[64h+f, 512c + n] = sum_k F[k, 64h+f] * P[k, 512c + n]

  lhsT = F [K=128, M=128]   stationary, block-diag: slot rows [0,64)
         (pool for the 4 "half A" tiles) carry feats in cols [0,64);
         rows [64,128) (half B pool) in cols [64,128).
  rhs  = P [K=128, N=512]   one-hot, built on DVE by a single
         tensor_scalar is_equal against a device-built iota:
         P[k, n] = (colof[k] == n), colof = 128*jj + cc encodes both
         the tile-within-half jj and the column cc; empty slots -1.
         (tensor_scalar, not a broadcast tensor_tensor: the [128,1]
         scalar operand is dtype-exempt, so the op runs in the DVE
         16-bit 2x mode.)

The matmul runs with is_transpose=True: P is a partial permutation
(at most one 1 per row AND per column), which is exactly the
generalized-transpose selection the PE transpose datapath performs,
and transpose mode writes PSUM in the lhsT dtype -- fp16.  fp16 PSUM
(a) halves PSUM bank usage (a [128, 1024] pair of chunks fits one
bank; transpose-mode writes may share a bank without the TRN2
multi-accumulation-group fault) and (b) makes the PSUM->SBUF copies
2-byte packed reads, so the DVE copies them in its 2x mode (~0.6us
per pair vs ~1.0us on Act).  Copies split ~2:1 Act:DVE; 10 dummy
warm-up matmuls ramp the PE p-state while input DMAs land.

Slots are pooled per (chunk, half): 4 tiles share 64 slots, so tile
occupancy only matters in aggregate (mean 43.6, cap 64).  The rare
overflowing group (1 in 1376 for seed 0) is fixed by swapping its
heaviest tile with the lightest tile of the lightest group; the
resulting tile permutation is applied on the host during unshard.

fp16 end-to-end: the one-hot matmul routes fp16 values exactly, so
the only error is the f32->fp16 input cast (~3.6e-4 max relative),
well under the 2e-2 gate, and memory traffic halves vs f32.

History: 194us (f32, per-tile matmuls) -> 88 (fp16 K-stacked chunks)
-> 61 (tensor_scalar 2x is_eq) -> 54 (balance/startup/tail tuning)
-> ~52 (fp16 PSUM via transpose mode).
"""

import numpy as np

import concourse.bass as bass
import concourse.tile as tile
from concourse import mybir
from concourse.bass_utils import run_bass_kernel_spmd

NUM_FEATURES = 64
MAX_CAV = 5
NX, NY = 704, 200
NUM_PIXELS = NY * NX            # 140800
TOTAL = MAX_CAV * NUM_PIXELS    # 704000
N_CORES = 8
CORE_COLS = TOTAL // N_CORES    # 88000 flat columns per core
TILE_COLS = 128
N_TILES = 688                   # 688*128 = 88064 >= 88000
CHUNKS = N_TILES // 8           # 86 chunks of 8 tiles
POOL = 64                       # slots shared by the 4 tiles of one half
OUT_W = CHUNKS * 512            # 44032

_PROG = None


def _split_excess_waits(nc, max_waits=1):
    """Walrus enforces tight per-instruction sync-wait encoding limits. Spill
    surplus waits onto single-wait EventSemaphore nops inserted just before
    the offending instruction on the same engine queue (same semantics:
    engine blocks at the nop, then proceeds)."""
    for blk in nc.main_func.blocks:
        i = 0
        while i < len(blk.instructions):
            inst = blk.instructions[i]
            si = inst.sync_info
            if si is None or len(si.on_wait) <= max_waits:
                i += 1
                continue
            waits = list(si.on_wait)
            keep, spill = waits[-max_waits:], waits[:-max_waits]
            for w in spill:
                nop = mybir.InstEventSemaphore(
                    name=f"I-{nc.next_id()}", ins=[], outs=[]
                )
                nop.engine = inst.engine
                nop.sync_info = mybir.SyncInfo(on_wait=[w], on_update=[])
                nc.register_instruction(nop)
                blk.instructions.insert(i, nop)
                i += 1
            si.on_wait = keep
            inst.sync_info = si
            i += 1


def _build_prog():
    f16 = mybir.dt.float16
    f32 = mybir.dt.float32
    nc = bass.Bass()
    feats = nc.dram_tensor("feats", [128, CHUNKS * 128], f16, kind="ExternalInput")
    colof = nc.dram_tensor("colof", [128, CHUNKS], f32, kind="ExternalInput")
    out = nc.dram_tensor("out", [128, OUT_W], f16, kind="ExternalOutput")

    with tile.TileContext(nc) as tc:
        with (
            tc.tile_pool(name="const", bufs=1) as constp,
            tc.tile_pool(name="pmat", bufs=8) as pmatp,
            tc.tile_pool(name="psum", bufs=6, space="PSUM") as psump,
            tc.tile_pool(name="wpsum", bufs=1, space="PSUM") as wpsump,
            tc.tile_pool(name="stage", bufs=4) as stagep,
        ):
            colof_sb = constp.tile([128, CHUNKS], f32)
            nc.sync.dma_start(colof_sb[:], colof[:])
            # iota built on-device (fp16 is exact for integers <= 2048)
            iota_sb = constp.tile([128, 512], f16)
            nc.gpsimd.iota(
                iota_sb[:], pattern=[[1, 512]], base=0,
                channel_multiplier=0,
                allow_small_or_imprecise_dtypes=True,
            )
            feats_sb = constp.tile([128, CHUNKS * 128], f16)
            lo = 0
            for n in (2, 14, 24, 46):       # chunks per feats DMA piece
                nc.sync.dma_start(
                    feats_sb[:, lo * 128:(lo + n) * 128],
                    feats[:, lo * 128:(lo + n) * 128],
                )
                lo += n

            # PE p-state warm-up: ~3us of back-to-back matmuls on the iota
            # tile while the input DMAs land, so real matmuls start at full
            # clock.  Results are discarded (bank reused by the main loop).
            wps = wpsump.tile([128, 1024], f32, space="PSUM")
            for w in range(10):
                nc.tensor.matmul(
                    out=wps[:, (w % 2) * 512:(w % 2) * 512 + 512],
                    lhsT=iota_sb[:, 0:128],
                    rhs=iota_sb[:],
                    start=True,
                    stop=True,
                )

            groups = [2, 6, 8] + [16] * 4 + [4, 2]
            c0 = 0
            qi = 0
            for n8 in groups:
                st = stagep.tile([128, 512 * n8], f16)
                for p0 in range(0, n8, 2):
                    np_ = min(2, n8 - p0)
                    # fp16 PSUM via the PE transpose path (P is a partial
                    # permutation, i.e. a generalized transpose selector);
                    # fp16 doubles bank capacity: one bank holds both
                    # chunks of a pair (transpose-mode writes share it)
                    ps = psump.tile([128, np_ * 512], f16, space="PSUM")
                    for j in range(p0, p0 + np_):
                        c = c0 + j
                        P = pmatp.tile([128, 512], f16)
                        nc.vector.tensor_scalar(
                            out=P[:],
                            in0=iota_sb[:],
                            scalar1=colof_sb[:, c:c + 1],
                            scalar2=None,
                            op0=mybir.AluOpType.is_equal,
                        )
                        nc.tensor.matmul(
                            out=ps[:, (j - p0) * 512:(j - p0 + 1) * 512],
                            lhsT=feats_sb[:, c * 128:(c + 1) * 128],
                            rhs=P[:],
                            start=True,
                            stop=True,
                            is_transpose=True,
                        )
                    # copy the pair in one contiguous instruction;
                    # fp16 PSUM reads get the DVE 2x mode, so DVE takes more
                    dst = st[:, p0 * 512:(p0 + np_) * 512]
                    src = ps[:]
                    if qi % 3 == 2 and c0 < CHUNKS - 16:
                        nc.vector.tensor_scalar_add(dst, src, 0.0)
                    else:
                        nc.scalar.activation(
                            dst, src, mybir.ActivationFunctionType.Copy
                        )
                    qi += 1
                nc.sync.dma_start(
                    out[:, c0 * 512:(c0 + n8) * 512], st[:]
                )
                c0 += n8
    _split_excess_waits(nc)
    return nc


def _host_prep(voxel_coords, pillar_features):
    vc = voxel_coords.astype(np.int64)
    flat = vc[:, 0] * NUM_PIXELS + vc[:, 2] * NX + vc[:, 3]
    feats = pillar_features.astype(np.float16)
    core = flat // CORE_COLS
    rem = flat - core * CORE_COLS
    t = rem // TILE_COLS            # tile within core, 0..687
    cc = rem - t * TILE_COLS        # column within tile

    in_maps = []
    perms = []
    for cidx in range(N_CORES):
        m = core == cidx
        tc_, cc_, fe_ = t[m], cc[m], feats[m]
        cnt = np.bincount(tc_, minlength=N_TILES)

        # perm[q] = original tile occupying virtual slot q; virtual slot q
        # belongs to chunk q//8, half (q%8)//4, jj q%4.
        perm = np.arange(N_TILES)
        gsum = cnt.reshape(N_TILES // 4, 4).sum(axis=1)
        for _ in range(64):
            gbad = int(np.argmax(gsum))
            if gsum[gbad] <= POOL:
                break
            glight = int(np.argmin(gsum))
            bt = gbad * 4 + int(np.argmax(cnt[perm[gbad * 4:gbad * 4 + 4]]))
            lt = glight * 4 + int(
                np.argmin(cnt[perm[glight * 4:glight * 4 + 4]])
            )
            perm[bt], perm[lt] = perm[lt], perm[bt]
            gsum[gbad] = cnt[perm[gbad * 4:gbad * 4 + 4]].sum()
            gsum[glight] = cnt[perm[glight * 4:glight * 4 + 4]].sum()
        assert gsum.max() <= POOL, f"group overflow: {gsum.max()}"
        perms.append(perm)

        pos = np.empty(N_TILES, np.int64)
        pos[perm] = np.arange(N_TILES)
        q = pos[tc_]                       # virtual tile slot per pillar
        chunk = q // 8
        h = (q % 8) // 4
        jj = q % 4
        grp = chunk * 2 + h                # slot pool id, 0..171

        # slot = rank of pillar within its pool
        order = np.argsort(grp, kind="stable")
        gs = grp[order]
        rank = np.arange(len(gs)) - np.searchsorted(gs, gs, side="left")
        slot = np.empty(len(gs), np.int64)
        slot[order] = rank
        assert slot.max() < POOL

        k = h * POOL + slot
        fa = np.zeros((128, CHUNKS, 2, 64), np.float16)
        fa[k, chunk, h, :] = fe_
        ca = np.full((128, CHUNKS), -1.0, np.float32)
        ca[k, chunk] = (jj * TILE_COLS + cc_).astype(np.float32)
        in_maps.append({
            "feats": fa.reshape(128, CHUNKS * 128),
            "colof": ca,
        })
    return in_maps, perms


def _unshard(core_outs, perms):
    full = np.empty((TOTAL, NUM_FEATURES), np.float32)
    for cidx, o in enumerate(core_outs):       # o: [128, OUT_W] fp16
        v = o.reshape(2, 64, CHUNKS, 4, 128)   # [h, f, chunk, jj, cc]
        v = v.transpose(2, 0, 3, 4, 1)         # [chunk, h, jj, cc, f]
        vt = v.reshape(N_TILES, TILE_COLS, NUM_FEATURES)
        ct = np.empty_like(vt)
        ct[perms[cidx]] = vt
        r = ct.reshape(N_TILES * TILE_COLS, NUM_FEATURES)[:CORE_COLS]
        full[cidx * CORE_COLS:(cidx + 1) * CORE_COLS] = r.astype(np.float32)
    return np.ascontiguousarray(
        full.reshape(MAX_CAV, NUM_PIXELS, NUM_FEATURES)
        .transpose(0, 2, 1)
        .reshape(MAX_CAV, NUM_FEATURES, NY, NX)
    )


def kernel(voxel_coords, pillar_features):
    global _PROG
    if _PROG is None:
        _PROG = _build_prog()
    in_maps, perms = _host_prep(voxel_coords, pillar_features)
    res = run_bass_kernel_spmd(_PROG, in_maps, list(range(N_CORES)))
    return _unshard([r["out"] for r in res.results], perms)


# revision 62
# speedup vs baseline: 1.0320x; 1.0302x over previous
"""PointPillarScatter on 8 TRN2 cores via PE one-hot matmul, fp16.

Scatter -> dense-matmul transform, one PE op per chunk of 8 canvas
tiles (tile = 128 consecutive canvas columns):

  out[64h+f, 512c + n] = sum_k F[k, 64h+f] * P[k, 512c + n]

  lhsT = F [K=128, M=128]   stationary, block-diag: slot rows [0,64)
         (pool for the 4 "half A" tiles) carry feats in cols [0,64);
         rows [64,128) (half B pool) in cols [64,128).
  rhs  = P [K=128, N=512]   one-hot, built on DVE by a single
         tensor_scalar is_equal against a device-built iota:
         P[k, n] = (colof[k] == n), colof = 128*jj + cc encodes both
         the tile-within-half jj and the column cc; empty slots -1.
         (tensor_scalar, not a broadcast tensor_tensor: the [128,1]
         scalar operand is dtype-exempt, so the op runs in the DVE
         16-bit 2x mode.)

The matmul runs with is_transpose=True: P is a partial permutation
(at most one 1 per row AND per column), which is exactly the
generalized-transpose selection the PE transpose datapath performs,
and transpose mode writes PSUM in the lhsT dtype -- fp16.  fp16 PSUM
(a) halves PSUM bank usage (a [128, 1024] pair of chunks fits one
bank; transpose-mode writes may share a bank without the TRN2
multi-accumulation-group fault) and (b) makes the PSUM->SBUF copies
2-byte packed reads, so the DVE copies them in its 2x mode (~0.6us
per pair vs ~1.0us on Act).  Copies split ~2:1 Act:DVE; 10 dummy
warm-up matmuls ramp the PE p-state while input DMAs land.

Slots are pooled per (chunk, half): 4 tiles share 64 slots, so tile
occupancy only matters in aggregate (mean 43.6, cap 64).  The rare
overflowing group (1 in 1376 for seed 0) is fixed by swapping its
heaviest tile with the lightest tile of the lightest group; the
resulting tile permutation is applied on the host during unshard.

fp16 end-to-end: the one-hot matmul routes fp16 values exactly, so
the only error is the f32->fp16 input cast (~3.6e-4 max relative),
well under the 2e-2 gate, and memory traffic halves vs f32.

History: 194us (f32, per-tile matmuls) -> 88 (fp16 K-stacked chunks)
-> 61 (tensor_scalar 2x is_eq) -> 54 (balance/startup/tail tuning)
-> ~52 (fp16 PSUM via transpose mode).
"""

import numpy as np

import concourse.bass as bass
import concourse.tile as tile
from concourse import mybir
from concourse.bass_utils import run_bass_kernel_spmd

NUM_FEATURES = 64
MAX_CAV = 5
NX, NY = 704, 200
NUM_PIXELS = NY * NX            # 140800
TOTAL = MAX_CAV * NUM_PIXELS    # 704000
N_CORES = 8
CORE_COLS = TOTAL // N_CORES    # 88000 flat columns per core
TILE_COLS = 128
N_TILES = 688                   # 688*128 = 88064 >= 88000
CHUNKS = N_TILES // 8           # 86 chunks of 8 tiles
POOL = 64                       # slots shared by the 4 tiles of one half
OUT_W = CHUNKS * 512            # 44032

_PROG = None


def _split_excess_waits(nc, max_waits=1):
    """Walrus enforces tight per-instruction sync-wait encoding limits. Spill
    surplus waits onto single-wait EventSemaphore nops inserted just before
    the offending instruction on the same engine queue (same semantics:
    engine blocks at the nop, then proceeds)."""
    for blk in nc.main_func.blocks:
        i = 0
        while i < len(blk.instructions):
            inst = blk.instructions[i]
            si = inst.sync_info
            if si is None or len(si.on_wait) <= max_waits:
                i += 1
                continue
            waits = list(si.on_wait)
            keep, spill = waits[-max_waits:], waits[:-max_waits]
            for w in spill:
                nop = mybir.InstEventSemaphore(
                    name=f"I-{nc.next_id()}", ins=[], outs=[]
                )
                nop.engine = inst.engine
                nop.sync_info = mybir.SyncInfo(on_wait=[w], on_update=[])
                nc.register_instruction(nop)
                blk.instructions.insert(i, nop)
                i += 1
            si.on_wait = keep
            inst.sync_info = si
            i += 1


def _build_prog():
    f16 = mybir.dt.float16
    f32 = mybir.dt.float32
    nc = bass.Bass()
    feats = nc.dram_tensor("feats", [128, CHUNKS * 128], f16, kind="ExternalInput")
    colof = nc.dram_tensor("colof", [128, CHUNKS], f32, kind="ExternalInput")
    out = nc.dram_tensor("out", [128, OUT_W], f16, kind="ExternalOutput")

    with tile.TileContext(nc) as tc:
        with (
            tc.tile_pool(name="const", bufs=1) as constp,
            tc.tile_pool(name="pmat", bufs=8) as pmatp,
            tc.tile_pool(name="psum", bufs=6, space="PSUM") as psump,
            tc.tile_pool(name="wpsum", bufs=1, space="PSUM") as wpsump,
            tc.tile_pool(name="stage", bufs=4) as stagep,
        ):
            colof_sb = constp.tile([128, CHUNKS], f32)
            nc.sync.dma_start(colof_sb[:], colof[:])
            # iota built on-device (fp16 is exact for integers <= 2048)
            iota_sb = constp.tile([128, 512], f16)
            nc.gpsimd.iota(
                iota_sb[:], pattern=[[1, 512]], base=0,
                channel_multiplier=0,
                allow_small_or_imprecise_dtypes=True,
            )
            feats_sb = constp.tile([128, CHUNKS * 128], f16)
            lo = 0
            for n in (2, 14, 24, 46):       # chunks per feats DMA piece
                nc.sync.dma_start(
                    feats_sb[:, lo * 128:(lo + n) * 128],
                    feats[:, lo * 128:(lo + n) * 128],
                )
                lo += n

            # PE p-state warm-up: ~3us of back-to-back matmuls on the iota
            # tile while the input DMAs land, so real matmuls start at full
            # clock.  Results are discarded (bank reused by the main loop).
            wps = wpsump.tile([128, 1024], f32, space="PSUM")
            for w in range(6):
                nc.tensor.matmul(
                    out=wps[:, (w % 2) * 512:(w % 2) * 512 + 512],
                    lhsT=iota_sb[:, 0:128],
                    rhs=iota_sb[:],
                    start=True,
                    stop=True,
                )

            groups = [2, 6] + [8] * 9 + [4, 2]
            c0 = 0
            qi = 0
            for n8 in groups:
                st = stagep.tile([128, 512 * n8], f16)
                for p0 in range(0, n8, 2):
                    np_ = min(2, n8 - p0)
                    # fp16 PSUM via the PE transpose path (P is a partial
                    # permutation, i.e. a generalized transpose selector);
                    # fp16 doubles bank capacity: one bank holds both
                    # chunks of a pair (transpose-mode writes share it)
                    ps = psump.tile([128, np_ * 512], f16, space="PSUM")
                    for j in range(p0, p0 + np_):
                        c = c0 + j
                        P = pmatp.tile([128, 512], f16)
                        nc.vector.tensor_scalar(
                            out=P[:],
                            in0=iota_sb[:],
                            scalar1=colof_sb[:, c:c + 1],
                            scalar2=None,
                            op0=mybir.AluOpType.is_equal,
                        )
                        nc.tensor.matmul(
                            out=ps[:, (j - p0) * 512:(j - p0 + 1) * 512],
                            lhsT=feats_sb[:, c * 128:(c + 1) * 128],
                            rhs=P[:],
                            start=True,
                            stop=True,
                            is_transpose=True,
                        )
                    # copy the pair in one contiguous instruction;
                    # fp16 PSUM reads get the DVE 2x mode, so DVE takes more
                    dst = st[:, p0 * 512:(p0 + np_) * 512]
                    src = ps[:]
                    if qi % 3 == 2 and c0 < CHUNKS - 16:
                        nc.vector.tensor_scalar_add(dst, src, 0.0)
                    else:
                        nc.scalar.activation(
                            dst, src, mybir.ActivationFunctionType.Copy
                        )
                    qi += 1
                nc.sync.dma_start(
                    out[:, c0 * 512:(c0 + n8) * 512], st[:]
                )
                c0 += n8
    _split_excess_waits(nc)
    return nc


def _host_prep(voxel_coords, pillar_features):
    vc = voxel_coords.astype(np.int64)
    flat = vc[:, 0] * NUM_PIXELS + vc[:, 2] * NX + vc[:, 3]
    feats = pillar_features.astype(np.float16)
    core = flat // CORE_COLS
    rem = flat - core * CORE_COLS
    t = rem // TILE_COLS            # tile within core, 0..687
    cc = rem - t * TILE_COLS        # column within tile

    in_maps = []
    perms = []
    for cidx in range(N_CORES):
        m = core == cidx
        tc_, cc_, fe_ = t[m], cc[m], feats[m]
        cnt = np.bincount(tc_, minlength=N_TILES)

        # perm[q] = original tile occupying virtual slot q; virtual slot q
        # belongs to chunk q//8, half (q%8)//4, jj q%4.
        perm = np.arange(N_TILES)
        gsum = cnt.reshape(N_TILES // 4, 4).sum(axis=1)
        for _ in range(64):
            gbad = int(np.argmax(gsum))
            if gsum[gbad] <= POOL:
                break
            glight = int(np.argmin(gsum))
            bt = gbad * 4 + int(np.argmax(cnt[perm[gbad * 4:gbad * 4 + 4]]))
            lt = glight * 4 + int(
                np.argmin(cnt[perm[glight * 4:glight * 4 + 4]])
            )
            perm[bt], perm[lt] = perm[lt], perm[bt]
            gsum[gbad] = cnt[perm[gbad * 4:gbad * 4 + 4]].sum()
            gsum[glight] = cnt[perm[glight * 4:glight * 4 + 4]].sum()
        assert gsum.max() <= POOL, f"group overflow: {gsum.max()}"
        perms.append(perm)

        pos = np.empty(N_TILES, np.int64)
        pos[perm] = np.arange(N_TILES)
        q = pos[tc_]                       # virtual tile slot per pillar
        chunk = q // 8
        h = (q % 8) // 4
        jj = q % 4
        grp = chunk * 2 + h                # slot pool id, 0..171

        # slot = rank of pillar within its pool
        order = np.argsort(grp, kind="stable")
        gs = grp[order]
        rank = np.arange(len(gs)) - np.searchsorted(gs, gs, side="left")
        slot = np.empty(len(gs), np.int64)
        slot[order] = rank
        assert slot.max() < POOL

        k = h * POOL + slot
        fa = np.zeros((128, CHUNKS, 2, 64), np.float16)
        fa[k, chunk, h, :] = fe_
        ca = np.full((128, CHUNKS), -1.0, np.float32)
        ca[k, chunk] = (jj * TILE_COLS + cc_).astype(np.float32)
        in_maps.append({
            "feats": fa.reshape(128, CHUNKS * 128),
            "colof": ca,
        })
    return in_maps, perms


def _unshard(core_outs, perms):
    full = np.empty((TOTAL, NUM_FEATURES), np.float32)
    for cidx, o in enumerate(core_outs):       # o: [128, OUT_W] fp16
        v = o.reshape(2, 64, CHUNKS, 4, 128)   # [h, f, chunk, jj, cc]
        v = v.transpose(2, 0, 3, 4, 1)         # [chunk, h, jj, cc, f]
        vt = v.reshape(N_TILES, TILE_COLS, NUM_FEATURES)
        ct = np.empty_like(vt)
        ct[perms[cidx]] = vt
        r = ct.reshape(N_TILES * TILE_COLS, NUM_FEATURES)[:CORE_COLS]
        full[cidx * CORE_COLS:(cidx + 1) * CORE_COLS] = r.astype(np.float32)
    return np.ascontiguousarray(
        full.reshape(MAX_CAV, NUM_PIXELS, NUM_FEATURES)
        .transpose(0, 2, 1)
        .reshape(MAX_CAV, NUM_FEATURES, NY, NX)
    )


def kernel(voxel_coords, pillar_features):
    global _PROG
    if _PROG is None:
        _PROG = _build_prog()
    in_maps, perms = _host_prep(voxel_coords, pillar_features)
    res = run_bass_kernel_spmd(_PROG, in_maps, list(range(N_CORES)))
    return _unshard([r["out"] for r in res.results], perms)
